# revision 4
# baseline (speedup 1.0000x reference)
"""Trainium2 Bass kernel for nn_DecoderRNN (LSTM decoder with big vocab projection).

Reference computation (T=64 steps, B=64, H=1024, CTX=1024, E=512, V=32000):
    h0 = tanh(context @ W_initS.T + b_initS); c0 likewise
    per step t:  x = [context, emb[seq[t]]]
                 gates = x @ W_ih.T + b_ih + h @ W_hh.T + b_hh
                 c' = sig(f)*c + sig(i)*tanh(g);  h' = sig(o)*tanh(c')
                 hid = tanh([h',c'] @ W_d1.T + b_d1)
                 out_t = hid @ W_d2.T + b_d2            # dominates FLOPs
    output: [T, B, V]

Sharding across 8 NeuronCores (one trn2 chip):
  - Recurrence is tensor-parallel over gate rows: core r owns H-chunk
    [128r, 128(r+1)) of i/f/o/g gates (reordered so one sigmoid covers
    i|f|o) and of h/c. Per step, an AllGather of the fp16 [h; c] chunk
    rebuilds the full state on every core.
  - The whole input projection Gx[t] = W_ih_shard @ [ctx; emb[seq[t]]].T
    (+ biases) is state-independent: hoisted into a pre-pass of N=512
    matmuls over all 64 steps, emitted interleaved with the first steps
    so the PE fills the early AllGather latency. Per step only one
    identity-matmul injects Gx[t] into the gates PSUM accumulation.
  - hid (d1) is TP-sharded per step-batch followed by an AllGather, then
    the vocab projection (d2) is V-sharded: core r computes
    out[:, 4000r:4000(r+1)] in fp16 with N=500 moving-dim matmuls,
    emitted paced between recurrence steps so the PE never starves
    (HAM stays warm) and never drains early. Final step-batches shrink
    (8,8,8,8,8,8,8,4,2,1,1) to cut the drain tail.

All matmuls run in fp16 (1 cycle/row on the PE vs fp32's 4); PSUM
accumulation and the LSTM cell state stay fp32. Output is written fp16
and upcast on the host (tolerance 2e-2 >> fp16 eps).
"""

import os
import time

import numpy as np

import concourse.bacc as bacc
import concourse.mybir as mybir
from concourse.tile import TileContext
from concourse.bass_utils import run_bass_kernel_spmd
from concourse.masks import make_identity

F16 = mybir.dt.float16
F32 = mybir.dt.float32
AF = mybir.ActivationFunctionType

R = 8                      # cores
V, E, H, CTX = 32000, 512, 1024, 1024
T, B = 64, 64
HC = H // R                # per-core H chunk (128)
VS = V // R                # per-core vocab shard (4000)
VC = 500                   # d2 moving-dim chunk (8 per shard)
KH = H // 128              # 8  k-tiles over H
KE = E // 128              # 4  k-tiles over E
KD1 = 2 * H // 128         # 16 k-tiles over [h;c]
# step-batches for d1/hid: large early, small at the end (drain tail)
BATCHES = [(0, 8), (8, 8), (16, 8), (24, 8), (32, 8), (40, 8), (48, 8),
           (56, 4), (60, 2), (62, 1), (63, 1)]

_CACHE = {}


def _build_program():
    """Build the SPMD Bass program (same on all cores; per-core data differs)."""
    nc = bacc.Bacc()

    # ---- kernel I/O ----------------------------------------------------
    ctx16 = nc.declare_dram_parameter("ctx16", [KH, 128, B], F16, isOutput=False)
    embt = nc.declare_dram_parameter("embt", [KE, 128, T * B], F16, isOutput=False)
    whh = nc.declare_dram_parameter("whh", [KH, 4, 128, 128], F16, isOutput=False)
    wihc = nc.declare_dram_parameter("wihc", [KH, 4, 128, 128], F16, isOutput=False)
    wihe = nc.declare_dram_parameter("wihe", [KE, 4, 128, 128], F16, isOutput=False)
    bg = nc.declare_dram_parameter("bg", [4, 128], F32, isOutput=False)
    binits = nc.declare_dram_parameter("binits", [128], F32, isOutput=False)
    binitc = nc.declare_dram_parameter("binitc", [128], F32, isOutput=False)
    winits = nc.declare_dram_parameter("winits", [KH, 128, 128], F16, isOutput=False)
    winitc = nc.declare_dram_parameter("winitc", [KH, 128, 128], F16, isOutput=False)
    wd1 = nc.declare_dram_parameter("wd1", [KD1, 128, 128], F16, isOutput=False)
    bd1 = nc.declare_dram_parameter("bd1", [128], F32, isOutput=False)
    wd2 = nc.declare_dram_parameter("wd2", [KH, 128, VS], F16, isOutput=False)
    bd2b = nc.declare_dram_parameter("bd2b", [128, VS], F16, isOutput=False)
    outp = nc.declare_dram_parameter("outp", [T * B, VS], F16, isOutput=True)

    # ---- internal DRAM (collective buffers) ----------------------------
    NB = len(BATCHES)
    hc_in = nc.dram_tensor("hc_in", [T + 1, 2 * 128, B], F16)
    hc_all = nc.dram_tensor("hc_all", [T + 1, 2 * H, B], F16, addr_space="Shared")
    hid_in = nc.dram_tensor("hid_in", [NB, 128, 512], F16)
    hid_all = nc.dram_tensor("hid_all", [NB, H, 512], F16, addr_space="Shared")
    rgroups = [list(range(R))]

    with TileContext(nc, num_cores=R) as tc:
        with (
            tc.tile_pool(name="const", bufs=1) as cpool,
            tc.tile_pool(name="work", bufs=3) as wpool,
            tc.tile_pool(name="ew", bufs=2) as epool,
            tc.tile_pool(name="gps", bufs=2, space="PSUM") as gates_pp,
            tc.tile_pool(name="d1ps", bufs=2, space="PSUM") as d1_pp,
            tc.tile_pool(name="d2ps", bufs=3, space="PSUM") as d2_pp,
        ):
            # ---- resident constants -----------------------------------
            whh_sb = cpool.tile([128, KH, 4, 128], F16)
            for k in range(KH):
                nc.sync.dma_start(out=whh_sb[:, k, :, :], in_=whh[k].rearrange("g p m -> p g m"))
            wihc_sb = cpool.tile([128, KH, 4, 128], F16)
            for k in range(KH):
                nc.sync.dma_start(out=wihc_sb[:, k, :, :], in_=wihc[k].rearrange("g p m -> p g m"))
            wihe_sb = cpool.tile([128, KE, 4, 128], F16)
            for k in range(KE):
                nc.sync.dma_start(out=wihe_sb[:, k, :, :], in_=wihe[k].rearrange("g p m -> p g m"))
            ctx_sb = cpool.tile([128, KH, B], F16)
            nc.sync.dma_start(out=ctx_sb[:], in_=ctx16.rearrange("k p b -> p k b"))
            winits_sb = cpool.tile([128, KH, 128], F16)
            nc.sync.dma_start(out=winits_sb[:], in_=winits.rearrange("k p m -> p k m"))
            winitc_sb = cpool.tile([128, KH, 128], F16)
            nc.sync.dma_start(out=winitc_sb[:], in_=winitc.rearrange("k p m -> p k m"))
            wd1_sb = cpool.tile([128, KD1, 128], F16)
            nc.sync.dma_start(out=wd1_sb[:], in_=wd1.rearrange("k p m -> p k m"))
            wd2_sb = cpool.tile([128, KH, VS], F16)
            for k in range(KH):
                nc.sync.dma_start(out=wd2_sb[:, k, :], in_=wd2[k])
            bd2b_sb = cpool.tile([128, VS], F16)
            nc.sync.dma_start(out=bd2b_sb[:], in_=bd2b[:])
            bg_sb = cpool.tile([128, 4], F32)
            nc.sync.dma_start(out=bg_sb[:], in_=bg.rearrange("g p -> p g"))
            binits_sb = cpool.tile([128, 1], F32)
            nc.sync.dma_start(out=binits_sb[:], in_=binits.rearrange("(p o) -> p o", o=1))
            binitc_sb = cpool.tile([128, 1], F32)
            nc.sync.dma_start(out=binitc_sb[:], in_=binitc.rearrange("(p o) -> p o", o=1))
            bd1_sb = cpool.tile([128, 1], F32)
            nc.sync.dma_start(out=bd1_sb[:], in_=bd1.rearrange("(p o) -> p o", o=1))
            ident = cpool.tile([128, 128], F16)
            make_identity(nc, ident[:])

            # ---- A_ctx = W_ihc_shard @ ctxT + b (bias folded here) ----
            actx_sb = cpool.tile([128, 4, B], F16)
            for g in range(4):
                ps = gates_pp.tile([128, 4 * B], F32, tag="gates")
                for k in range(KH):
                    nc.tensor.matmul(
                        ps[:, :B], wihc_sb[:, k, g, :], ctx_sb[:, k, :],
                        start=(k == 0), stop=(k == KH - 1),
                    )
                nc.vector.tensor_scalar_add(actx_sb[:, g, :], ps[:, :B],
                                            bg_sb[:, g:g + 1])
            # replicate actx 8x along t for the Gx evacuation add
            actx_rep = cpool.tile([128, 4, 8, B], F16)
            for g in range(4):
                for s in range(8):
                    nc.vector.tensor_copy(out=actx_rep[:, g, s, :],
                                          in_=actx_sb[:, g, :])

            # ---- h0 / c0 chunks ---------------------------------------
            ps = gates_pp.tile([128, 4 * B], F32, tag="gates")
            for k in range(KH):
                nc.tensor.matmul(ps[:, :B], winits_sb[:, k, :], ctx_sb[:, k, :],
                                 start=(k == 0), stop=(k == KH - 1))
            h16 = epool.tile([128, B], F16, tag="h16")
            nc.scalar.activation(h16[:], ps[:, :B], AF.Tanh, bias=binits_sb[:])
            ps = gates_pp.tile([128, 4 * B], F32, tag="gates")
            for k in range(KH):
                nc.tensor.matmul(ps[:, :B], winitc_sb[:, k, :], ctx_sb[:, k, :],
                                 start=(k == 0), stop=(k == KH - 1))
            c_loc = epool.tile([128, B], F32, tag="cloc")
            nc.scalar.activation(c_loc[:], ps[:, :B], AF.Tanh, bias=binitc_sb[:])
            c16 = epool.tile([128, B], F16, tag="c16")
            nc.vector.tensor_copy(out=c16[:], in_=c_loc[:])

            nc.sync.dma_start(out=hc_in[0, 0:128, :], in_=h16[:])
            nc.sync.dma_start(out=hc_in[0, 128:256, :], in_=c16[:])
            nc.gpsimd.collective_compute(
                "AllGather", mybir.AluOpType.bypass,
                ins=[hc_in[0]], outs=[hc_all[0]], replica_groups=rgroups,
            )

            # ---- hoisted input projection: Gx[t] for all t ------------
            # gxe_tiles[c] holds Gx for steps 8c..8c+7: [128, 8, 4, B] f16
            gxe_tiles = [cpool.tile([128, 8, 4, B], F16, name=f"gxe{c}")
                         for c in range(8)]

            def emit_gx_chunk(g, c):
                # one gate, 8 steps: psum [128, 512] = W_ihe_g @ embT chunk
                ps2 = d2_pp.tile([128, 512], F32, tag="d2")
                et = wpool.tile([128, KE, 512], F16, tag="embt")
                nc.sync.dma_start(
                    out=et[:], in_=embt[:, :, c * 512:(c + 1) * 512]
                    .rearrange("k p n -> p k n"))
                for k in range(KE):
                    nc.tensor.matmul(ps2[:], wihe_sb[:, k, g, :], et[:, k, :],
                                     start=(k == 0), stop=(k == KE - 1))
                # evacuate with the (bias-folded) context part added in
                nc.vector.tensor_tensor(
                    out=gxe_tiles[c][:, :, g, :],
                    in0=ps2[:].rearrange("p (s b) -> p s b", s=8),
                    in1=actx_rep[:, g, :, :],
                    op=mybir.AluOpType.add,
                )

            # chunk 0 must exist before step 0 reads it; the rest are paced
            # into the early steps' AllGather-wait windows
            for g in range(4):
                emit_gx_chunk(g, 0)
            gxq = [(g, c) for c in range(1, 8) for g in range(4)]

            # ---- d2 work queue ----------------------------------------
            d2q = []
            _hid_sb = {}

            def emit_d2_unit(j, m, mrows, vc):
                ps2 = d2_pp.tile([128, VC], F32, tag="d2")
                hidT, cols0 = _hid_sb[j]
                for k in range(KH):
                    nc.tensor.matmul(
                        ps2[:mrows, :], hidT[:, k, m * 128:m * 128 + mrows],
                        wd2_sb[:, k, vc * VC:(vc + 1) * VC],
                        start=(k == 0), stop=(k == KH - 1),
                    )
                osb = wpool.tile([128, VC], F16, tag="outsb")
                nc.vector.tensor_tensor(
                    out=osb[:mrows, :], in0=ps2[:mrows, :],
                    in1=bd2b_sb[:mrows, vc * VC:(vc + 1) * VC],
                    op=mybir.AluOpType.add,
                )
                r0 = cols0 + m * 128
                nc.sync.dma_start(
                    out=outp[r0:r0 + mrows, vc * VC:(vc + 1) * VC],
                    in_=osb[:mrows, :],
                )

            def emit_filler(nmax):
                done = 0
                while done < nmax and gxq:
                    emit_gx_chunk(*gxq.pop(0))
                    done += 1
                while done < nmax and d2q:
                    emit_d2_unit(*d2q.pop(0))
                    done += 1

            # ---- main recurrence loop ---------------------------------
            bi = 0  # next batch index
            for t in range(T):
                # full h_t from the AllGather (single strided DMA)
                hT = wpool.tile([128, KH, B], F16, tag="hT")
                nc.sync.dma_start(
                    out=hT[:],
                    in_=hc_all[t].rearrange("(q s p) b -> p s q b", s=2, p=128)[:, 0, :, :],
                )

                ps = gates_pp.tile([128, 4 * B], F32, tag="gates")
                # inject Gx[t] (includes ctx part + biases), then accumulate whh
                nc.tensor.matmul(
                    ps[:], ident[:],
                    gxe_tiles[t // 8][:, t % 8, :, :].rearrange("p g b -> p (g b)"),
                    start=True, stop=False)
                for g in range(4):
                    sl = ps[:, g * B:(g + 1) * B]
                    for k in range(KH):
                        nc.tensor.matmul(sl, whh_sb[:, k, g, :], hT[:, k, :],
                                         start=False,
                                         stop=(g == 3 and k == KH - 1))

                # pointwise: gate order is [i|f|o|g]
                sg3 = epool.tile([128, 3 * B], F32, tag="sg3")
                nc.scalar.activation(sg3[:], ps[:, 0:3 * B], AF.Sigmoid)
                tang = epool.tile([128, B], F32, tag="tang")
                nc.scalar.activation(tang[:], ps[:, 3 * B:4 * B], AF.Tanh)
                t1 = epool.tile([128, B], F32, tag="t1")
                nc.vector.tensor_tensor(out=t1[:], in0=sg3[:, B:2 * B], in1=c_loc[:],
                                        op=mybir.AluOpType.mult)
                t2 = epool.tile([128, B], F32, tag="t2")
                nc.vector.tensor_tensor(out=t2[:], in0=sg3[:, 0:B], in1=tang[:],
                                        op=mybir.AluOpType.mult)
                c_loc = epool.tile([128, B], F32, tag="cloc")
                nc.vector.tensor_tensor(out=c_loc[:], in0=t1[:], in1=t2[:],
                                        op=mybir.AluOpType.add)
                tanc = epool.tile([128, B], F32, tag="tanc")
                nc.scalar.activation(tanc[:], c_loc[:], AF.Tanh)
                hc16 = epool.tile([128, 2, B], F16, tag="hc16")
                nc.vector.tensor_tensor(out=hc16[:, 0, :], in0=sg3[:, 2 * B:3 * B],
                                        in1=tanc[:], op=mybir.AluOpType.mult)
                nc.vector.tensor_copy(out=hc16[:, 1, :], in_=c_loc[:])

                nc.sync.dma_start(
                    out=hc_in[t + 1].rearrange("(s p) b -> p s b", p=128),
                    in_=hc16[:])
                nc.gpsimd.collective_compute(
                    "AllGather", mybir.AluOpType.bypass,
                    ins=[hc_in[t + 1]], outs=[hc_all[t + 1]],
                    replica_groups=rgroups,
                )

                # batch boundary: d1 + hid AllGather, then queue d2 units
                if bi < NB and t == BATCHES[bi][0] + BATCHES[bi][1] - 1:
                    s0, ns = BATCHES[bi]
                    ncols = ns * B
                    psd1 = d1_pp.tile([128, 512], F32, tag="d1")
                    for k in range(KD1):
                        rhs = wpool.tile([128, 8 * B], F16, tag="d1rhs")
                        nc.sync.dma_start(
                            out=rhs[:, :ncols],
                            in_=hc_all[s0 + 1:s0 + ns + 1,
                                       k * 128:(k + 1) * 128, :].rearrange(
                                       "s p b -> p s b"),
                        )
                        nc.tensor.matmul(psd1[:, :ncols], wd1_sb[:, k, :],
                                         rhs[:, :ncols],
                                         start=(k == 0), stop=(k == KD1 - 1))
                    hloc = wpool.tile([128, 512], F16, tag="hloc")
                    nc.scalar.activation(hloc[:, :ncols], psd1[:, :ncols],
                                         AF.Tanh, bias=bd1_sb[:])
                    nc.sync.dma_start(out=hid_in[bi, :, :ncols],
                                      in_=hloc[:, :ncols])
                    nc.gpsimd.collective_compute(
                        "AllGather", mybir.AluOpType.bypass,
                        ins=[hid_in[bi]], outs=[hid_all[bi]],
                        replica_groups=rgroups,
                    )
                    hsb = wpool.tile([128, KH, 512], F16, tag="hidT", bufs=2)
                    nc.sync.dma_start(
                        out=hsb[:, :, :ncols],
                        in_=hid_all[bi, :, :ncols].rearrange(
                            "(k p) n -> p k n", p=128),
                    )
                    _hid_sb[bi] = (hsb, s0 * B)
                    for m in range((ncols + 127) // 128):
                        mrows = min(128, ncols - m * 128)
                        for vc in range(VS // VC):
                            d2q.append((bi, m, mrows, vc))
                    bi += 1

                # paced filler: ~4.5 units/step so the queue neither
                # starves the PE nor drains before the last batches land
                emit_filler(5 if t % 2 else 4)

            while gxq:
                emit_gx_chunk(*gxq.pop(0))
            while d2q:
                emit_d2_unit(*d2q.pop(0))

    nc.finalize()
    return nc


GATE_ORDER = [0, 1, 3, 2]  # reference i,f,g,o -> kernel [i|f|o|g]


def _prep_inputs(seq, context, emb, W_ih, b_ih, W_hh, b_hh, W_initS, b_initS,
                 W_initC, b_initC, W_d1, b_d1, W_d2, b_d2):
    """Host-side layout prep: transposes, fp16 casts, per-core sharding."""
    f16, f32 = np.float16, np.float32
    seq = np.asarray(seq)
    context = np.asarray(context, f32)
    emb = np.asarray(emb, f32)

    # emb[seq].T all steps: [KE, 128, T*B] (column index = t*B + b)
    g = emb[seq.reshape(-1)].reshape(T * B, E).T        # [E, T*B]
    embt = np.ascontiguousarray(g.reshape(KE, 128, T * B)).astype(f16)

    ctxT = np.ascontiguousarray(context.T)              # [CTX, B]
    ctx16 = ctxT.reshape(KH, 128, B).astype(f16)

    bsum = (np.asarray(b_ih, f32) + np.asarray(b_hh, f32))  # [4H]

    W_ihc = np.asarray(W_ih, f32)[:, :CTX]              # [4H, CTX]
    W_ihe = np.asarray(W_ih, f32)[:, CTX:]              # [4H, E]
    W_hh = np.asarray(W_hh, f32)
    W_d1 = np.asarray(W_d1, f32)
    W_d2 = np.asarray(W_d2, f32)

    # d1 row permutation to match AllGather layout [h_q; c_q interleaved]
    perm = np.empty(2 * H, np.int64)
    for q in range(R):
        perm[256 * q:256 * q + 128] = np.arange(128 * q, 128 * (q + 1))
        perm[256 * q + 128:256 * (q + 1)] = H + np.arange(128 * q, 128 * (q + 1))
    W_d1T_perm = W_d1.T[perm, :]                        # [2H, H]

    maps = []
    for r in range(R):
        rows = lambda g_: slice(1024 * g_ + 128 * r, 1024 * g_ + 128 * (r + 1))

        def gate_tiles(W, KT):
            # [KT, 4, 128(k), 128(m)]: W rows = gate-chunk rows of core r,
            # gates reordered to [i|f|o|g]
            a = np.empty((KT, 4, 128, 128), f32)
            for gi in range(4):
                Wg = W[rows(GATE_ORDER[gi])]            # [128, KT*128]
                a[:, gi] = Wg.reshape(128, KT, 128).transpose(1, 2, 0)
            return a.astype(f16)

        whh_r = gate_tiles(W_hh, KH)
        wihc_r = gate_tiles(W_ihc, KH)
        wihe_r = gate_tiles(W_ihe, KE)
        bg_r = np.stack([bsum[rows(GATE_ORDER[gi])]
                         for gi in range(4)]).astype(f32)  # [4,128]

        hcrows = slice(128 * r, 128 * (r + 1))
        winits_r = np.ascontiguousarray(
            np.asarray(W_initS, f32)[hcrows].T.reshape(KH, 128, 128)).astype(f16)
        winitc_r = np.ascontiguousarray(
            np.asarray(W_initC, f32)[hcrows].T.reshape(KH, 128, 128)).astype(f16)
        binits_r = np.asarray(b_initS, f32)[hcrows].copy()
        binitc_r = np.asarray(b_initC, f32)[hcrows].copy()

        wd1_r = np.ascontiguousarray(
            W_d1T_perm[:, hcrows].reshape(KD1, 128, 128)).astype(f16)
        bd1_r = np.asarray(b_d1, f32)[hcrows].copy()

        vsl = slice(VS * r, VS * (r + 1))
        wd2_r = np.ascontiguousarray(
            W_d2[vsl].T.reshape(KH, 128, VS)).astype(f16)
        bd2b_r = np.broadcast_to(
            np.asarray(b_d2, f32)[vsl], (128, VS)).astype(f16).copy()

        maps.append({
            "ctx16": ctx16, "embt": embt,
            "whh": whh_r, "wihc": wihc_r, "wihe": wihe_r, "bg": bg_r,
            "binits": binits_r, "binitc": binitc_r,
            "winits": winits_r, "winitc": winitc_r,
            "wd1": wd1_r, "bd1": bd1_r,
            "wd2": wd2_r, "bd2b": bd2b_r,
        })
    return maps


def kernel(**inputs):
    inputs.pop("mode", None)
    in_maps = _prep_inputs(**{k: np.asarray(v) for k, v in inputs.items()})
    if "nc" not in _CACHE:
        _CACHE["nc"] = _build_program()
    res = run_bass_kernel_spmd(_CACHE["nc"], in_maps, list(range(R)))
    _CACHE["last_res"] = res
    if getattr(res, "exec_time_ns", None):
        print(f"[profile] exec_time_ns: {res.exec_time_ns}")
    shards = [res.results[r]["outp"] for r in range(R)]       # each [T*B, VS] f16
    out = np.concatenate(shards, axis=1).astype(np.float32)   # [T*B, V]
    return out.reshape(T, B, V)


def timed_runs(inputs, n=6):
    """Test-only helper: execute the compiled program n times on device-
    resident inputs and return per-iteration wall times (seconds)."""
    import jax
    import jax.numpy as jnp
    from jax.sharding import Mesh, PartitionSpec, NamedSharding
    from jax.experimental.shard_map import shard_map
    from concourse import bass2jax
    import concourse.mybir as mybir_

    inputs = {k: np.asarray(v) for k, v in inputs.items()}
    inputs.pop("mode", None)
    in_maps = _prep_inputs(**inputs)
    if "nc" not in _CACHE:
        _CACHE["nc"] = _build_program()
    nc = _CACHE["nc"]
    bass2jax.install_neuronx_cc_hook()

    partition_name = nc.partition_id_tensor.name if nc.partition_id_tensor else None
    in_names, out_names, out_avals = [], [], []
    for alloc in nc.m.functions[0].allocations:
        if not isinstance(alloc, mybir_.MemoryLocationSet):
            continue
        name = alloc.memorylocations[0].name
        if alloc.kind == "ExternalInput":
            if name != partition_name:
                in_names.append(name)
        elif alloc.kind == "ExternalOutput":
            out_names.append(name)
            out_avals.append(
                jax.core.ShapedArray(tuple(alloc.tensor_shape),
                                     mybir_.dt.np(alloc.dtype)))

    all_in_names = in_names + out_names
    if partition_name is not None:
        all_in_names = all_in_names + [partition_name]

    def _body(*args):
        operands = list(args)
        if partition_name is not None:
            operands.append(bass2jax.partition_id_tensor())
        outs = bass2jax._bass_exec_p.bind(
            *operands, out_avals=tuple(out_avals),
            in_names=tuple(all_in_names),
            out_names=tuple(out_names),
            lowering_input_output_aliases=(),
            sim_require_finite=True, sim_require_nnan=True, nc=nc,
        )
        return tuple(outs)

    devices = jax.devices()[:R]
    mesh = Mesh(np.asarray(devices), ("core",))
    nspec = (PartitionSpec("core"),) * (len(in_names) + len(out_names))
    sharded = jax.jit(shard_map(_body, mesh=mesh, in_specs=nspec,
                                out_specs=(PartitionSpec("core"),) * len(out_names),
                                check_rep=False), keep_unused=True)

    concat_in = [
        jax.device_put(
            np.concatenate([np.asarray(in_maps[c][nm]) for c in range(R)], axis=0),
            NamedSharding(mesh, PartitionSpec("core")))
        for nm in in_names
    ]
    zero_fn = jax.jit(
        lambda: tuple(
            jnp.zeros((R * av.shape[0], *av.shape[1:]), av.dtype)
            for av in out_avals),
        out_shardings=tuple(NamedSharding(mesh, PartitionSpec("core"))
                            for _ in out_avals))
    zeros = [jax.block_until_ready(z) for z in zero_fn()]

    times = []
    for _ in range(n):
        t0 = time.time()
        outs = sharded(*concat_in, *zeros)
        jax.block_until_ready(outs)
        times.append(time.time() - t0)
    return times


if __name__ == "__main__":
    rng = np.random.default_rng(0)
    ins = {
        "seq": rng.integers(0, V, (T, B)).astype(np.int32),
        "context": rng.standard_normal((B, CTX)).astype(np.float32),
        "emb": (rng.standard_normal((V, E)) * 0.02).astype(np.float32),
        "W_ih": (rng.standard_normal((4 * H, E + CTX)) / np.sqrt(E + CTX)).astype(np.float32),
        "b_ih": np.zeros(4 * H, np.float32),
        "W_hh": (rng.standard_normal((4 * H, H)) / np.sqrt(H)).astype(np.float32),
        "b_hh": np.zeros(4 * H, np.float32),
        "W_initS": (rng.standard_normal((H, CTX)) / np.sqrt(CTX)).astype(np.float32),
        "b_initS": np.zeros(H, np.float32),
        "W_initC": (rng.standard_normal((H, CTX)) / np.sqrt(CTX)).astype(np.float32),
        "b_initC": np.zeros(H, np.float32),
        "W_d1": (rng.standard_normal((H, 2 * H)) / np.sqrt(2 * H)).astype(np.float32),
        "b_d1": np.zeros(H, np.float32),
        "W_d2": (rng.standard_normal((V, H)) / np.sqrt(H)).astype(np.float32),
        "b_d2": np.zeros(V, np.float32),
        "mode": 1,
    }
    out = kernel(**ins)
    print("kernel output", out.shape, out.dtype, float(np.abs(out).max()))


# revision 7
# speedup vs baseline: 2.2640x; 2.2640x over previous
"""Trainium2 Bass kernel for nn_DecoderRNN (LSTM decoder with big vocab projection).

Reference computation (T=64 steps, B=64, H=1024, CTX=1024, E=512, V=32000):
    h0 = tanh(context @ W_initS.T + b_initS); c0 likewise
    per step t:  x = [context, emb[seq[t]]]
                 gates = x @ W_ih.T + b_ih + h @ W_hh.T + b_hh
                 c' = sig(f)*c + sig(i)*tanh(g);  h' = sig(o)*tanh(c')
                 hid = tanh([h',c'] @ W_d1.T + b_d1)
                 out_t = hid @ W_d2.T + b_d2            # dominates FLOPs
    output: [T, B, V]

Sharding across 8 NeuronCores (one trn2 chip):
  - Recurrence is tensor-parallel over gate rows: core r owns H-chunk
    [128r, 128(r+1)) of i/f/o/g gates (reordered so one sigmoid covers
    i|f|o) and of h/c. Per step, an AllGather of the fp16 [h; c] chunk
    rebuilds the full state on every core.
  - The whole input projection Gx[t] = W_ih_shard @ [ctx; emb[seq[t]]].T
    (+ biases) is state-independent: hoisted into a pre-pass of N=512
    matmuls over all 64 steps, emitted interleaved with the first steps
    so the PE fills the early AllGather latency. Per step only one
    identity-matmul injects Gx[t] into the gates PSUM accumulation.
  - hid (d1) is TP-sharded per step-batch followed by an AllGather, then
    the vocab projection (d2) is V-sharded: core r computes
    out[:, 4000r:4000(r+1)] in fp16 with N=500 moving-dim matmuls,
    emitted paced between recurrence steps so the PE never starves
    (HAM stays warm) and never drains early. Final step-batches shrink
    (8,8,8,8,8,8,8,4,2,1,1) to cut the drain tail.

All matmuls run in fp16 (1 cycle/row on the PE vs fp32's 4); PSUM
accumulation and the LSTM cell state stay fp32. Output is written fp16
and upcast on the host (tolerance 2e-2 >> fp16 eps).
"""

import os
import time

import numpy as np

import concourse.bacc as bacc
import concourse.mybir as mybir
from concourse.tile import TileContext
from concourse.bass_utils import run_bass_kernel_spmd
from concourse.masks import make_identity

F16 = mybir.dt.float16
F32 = mybir.dt.float32
AF = mybir.ActivationFunctionType

R = 8                      # cores
V, E, H, CTX = 32000, 512, 1024, 1024
T, B = 64, 64
HC = H // R                # per-core H chunk (128)
VS = V // R                # per-core vocab shard (4000)
VC = 500                   # d2 moving-dim chunk (8 per shard)
KH = H // 128              # 8  k-tiles over H
KE = E // 128              # 4  k-tiles over E
KD1 = 2 * H // 128         # 16 k-tiles over [h;c]
# step-batches for d1/hid: large early, small at the end (drain tail)
BATCHES = [(0, 8), (8, 8), (16, 8), (24, 8), (32, 8), (40, 8), (48, 8),
           (56, 4), (60, 2), (62, 1), (63, 1)]

_CACHE = {}


def _build_program():
    """Build the SPMD Bass program (same on all cores; per-core data differs)."""
    nc = bacc.Bacc()

    # ---- kernel I/O ----------------------------------------------------
    ctx16 = nc.declare_dram_parameter("ctx16", [KH, 128, B], F16, isOutput=False)
    embt = nc.declare_dram_parameter("embt", [KE, 128, T * B], F16, isOutput=False)
    whh = nc.declare_dram_parameter("whh", [KH, 4, 128, 128], F16, isOutput=False)
    wihc = nc.declare_dram_parameter("wihc", [KH, 4, 128, 128], F16, isOutput=False)
    wihe = nc.declare_dram_parameter("wihe", [KE, 4, 128, 128], F16, isOutput=False)
    bg = nc.declare_dram_parameter("bg", [4, 128], F32, isOutput=False)
    binits = nc.declare_dram_parameter("binits", [128], F32, isOutput=False)
    binitc = nc.declare_dram_parameter("binitc", [128], F32, isOutput=False)
    winits = nc.declare_dram_parameter("winits", [KH, 128, 128], F16, isOutput=False)
    winitc = nc.declare_dram_parameter("winitc", [KH, 128, 128], F16, isOutput=False)
    wd1 = nc.declare_dram_parameter("wd1", [KD1, 128, 128], F16, isOutput=False)
    bd1 = nc.declare_dram_parameter("bd1", [128], F32, isOutput=False)
    wd2 = nc.declare_dram_parameter("wd2", [KH, 128, VS], F16, isOutput=False)
    bd2b = nc.declare_dram_parameter("bd2b", [128, VS], F16, isOutput=False)
    outp = nc.declare_dram_parameter("outp", [T * B, VS], F16, isOutput=True)

    # ---- internal DRAM (collective buffers) ----------------------------
    NB = len(BATCHES)
    hc_in = nc.dram_tensor("hc_in", [T + 1, 2 * 128, B], F16)
    hc_all = nc.dram_tensor("hc_all", [T + 1, 2 * H, B], F16, addr_space="Shared")
    hid_in = nc.dram_tensor("hid_in", [NB, 128, 512], F16)
    hid_all = nc.dram_tensor("hid_all", [NB, H, 512], F16, addr_space="Shared")
    rgroups = [list(range(R))]

    with TileContext(nc, num_cores=R) as tc:
        with (
            tc.tile_pool(name="const", bufs=1) as cpool,
            tc.tile_pool(name="work", bufs=3) as wpool,
            tc.tile_pool(name="ew", bufs=2) as epool,
            tc.tile_pool(name="gps", bufs=2, space="PSUM") as gates_pp,
            tc.tile_pool(name="d1ps", bufs=2, space="PSUM") as d1_pp,
            tc.tile_pool(name="d2ps", bufs=3, space="PSUM") as d2_pp,
        ):
            # ---- resident constants -----------------------------------
            whh_sb = cpool.tile([128, KH, 4, 128], F16)
            for k in range(KH):
                nc.sync.dma_start(out=whh_sb[:, k, :, :], in_=whh[k].rearrange("g p m -> p g m"))
            wihc_sb = cpool.tile([128, KH, 4, 128], F16)
            for k in range(KH):
                nc.sync.dma_start(out=wihc_sb[:, k, :, :], in_=wihc[k].rearrange("g p m -> p g m"))
            wihe_sb = cpool.tile([128, KE, 4, 128], F16)
            for k in range(KE):
                nc.sync.dma_start(out=wihe_sb[:, k, :, :], in_=wihe[k].rearrange("g p m -> p g m"))
            ctx_sb = cpool.tile([128, KH, B], F16)
            nc.sync.dma_start(out=ctx_sb[:], in_=ctx16.rearrange("k p b -> p k b"))
            winits_sb = cpool.tile([128, KH, 128], F16)
            nc.sync.dma_start(out=winits_sb[:], in_=winits.rearrange("k p m -> p k m"))
            winitc_sb = cpool.tile([128, KH, 128], F16)
            nc.sync.dma_start(out=winitc_sb[:], in_=winitc.rearrange("k p m -> p k m"))
            wd1_sb = cpool.tile([128, KD1, 128], F16)
            nc.sync.dma_start(out=wd1_sb[:], in_=wd1.rearrange("k p m -> p k m"))
            wd2_sb = cpool.tile([128, KH, VS], F16)
            for k in range(KH):
                nc.sync.dma_start(out=wd2_sb[:, k, :], in_=wd2[k])
            bd2b_sb = cpool.tile([128, VS], F16)
            nc.sync.dma_start(out=bd2b_sb[:], in_=bd2b[:])
            bg_sb = cpool.tile([128, 4], F32)
            nc.sync.dma_start(out=bg_sb[:], in_=bg.rearrange("g p -> p g"))
            binits_sb = cpool.tile([128, 1], F32)
            nc.sync.dma_start(out=binits_sb[:], in_=binits.rearrange("(p o) -> p o", o=1))
            binitc_sb = cpool.tile([128, 1], F32)
            nc.sync.dma_start(out=binitc_sb[:], in_=binitc.rearrange("(p o) -> p o", o=1))
            bd1_sb = cpool.tile([128, 1], F32)
            nc.sync.dma_start(out=bd1_sb[:], in_=bd1.rearrange("(p o) -> p o", o=1))
            ident = cpool.tile([128, 128], F16)
            make_identity(nc, ident[:])

            # ---- A_ctx = W_ihc_shard @ ctxT + b (bias folded here) ----
            actx_sb = cpool.tile([128, 4, B], F16)
            for g in range(4):
                ps = gates_pp.tile([128, 4 * B], F32, tag="gates")
                for k in range(KH):
                    nc.tensor.matmul(
                        ps[:, :B], wihc_sb[:, k, g, :], ctx_sb[:, k, :],
                        start=(k == 0), stop=(k == KH - 1),
                    )
                nc.vector.tensor_scalar_add(actx_sb[:, g, :], ps[:, :B],
                                            bg_sb[:, g:g + 1])
            # replicate actx 8x along t for the Gx evacuation add
            actx_rep = cpool.tile([128, 4, 8, B], F16)
            for g in range(4):
                for s in range(8):
                    nc.vector.tensor_copy(out=actx_rep[:, g, s, :],
                                          in_=actx_sb[:, g, :])

            # ---- h0 / c0 chunks ---------------------------------------
            ps = gates_pp.tile([128, 4 * B], F32, tag="gates")
            for k in range(KH):
                nc.tensor.matmul(ps[:, :B], winits_sb[:, k, :], ctx_sb[:, k, :],
                                 start=(k == 0), stop=(k == KH - 1))
            h16 = epool.tile([128, B], F16, tag="h16")
            nc.scalar.activation(h16[:], ps[:, :B], AF.Tanh, bias=binits_sb[:])
            ps = gates_pp.tile([128, 4 * B], F32, tag="gates")
            for k in range(KH):
                nc.tensor.matmul(ps[:, :B], winitc_sb[:, k, :], ctx_sb[:, k, :],
                                 start=(k == 0), stop=(k == KH - 1))
            c_loc = epool.tile([128, B], F32, tag="cloc")
            nc.scalar.activation(c_loc[:], ps[:, :B], AF.Tanh, bias=binitc_sb[:])
            c16 = epool.tile([128, B], F16, tag="c16")
            nc.vector.tensor_copy(out=c16[:], in_=c_loc[:])

            nc.sync.dma_start(out=hc_in[0, 0:128, :], in_=h16[:])
            nc.sync.dma_start(out=hc_in[0, 128:256, :], in_=c16[:])
            nc.gpsimd.collective_compute(
                "AllGather", mybir.AluOpType.bypass,
                ins=[hc_in[0]], outs=[hc_all[0]], replica_groups=rgroups,
            )

            # ---- hoisted input projection: Gx[t] for all t ------------
            # gxe_tiles[c] holds Gx for steps 8c..8c+7: [128, 8, 4, B] f16
            gxe_tiles = [cpool.tile([128, 8, 4, B], F16, name=f"gxe{c}")
                         for c in range(8)]

            def emit_gx_chunk(g, c):
                # one gate, 8 steps: psum [128, 512] = W_ihe_g @ embT chunk
                ps2 = d2_pp.tile([128, 512], F32, tag="d2")
                et = wpool.tile([128, KE, 512], F16, tag="embt")
                nc.sync.dma_start(
                    out=et[:], in_=embt[:, :, c * 512:(c + 1) * 512]
                    .rearrange("k p n -> p k n"))
                for k in range(KE):
                    nc.tensor.matmul(ps2[:], wihe_sb[:, k, g, :], et[:, k, :],
                                     start=(k == 0), stop=(k == KE - 1))
                # evacuate with the (bias-folded) context part added in
                nc.vector.tensor_tensor(
                    out=gxe_tiles[c][:, :, g, :],
                    in0=ps2[:].rearrange("p (s b) -> p s b", s=8),
                    in1=actx_rep[:, g, :, :],
                    op=mybir.AluOpType.add,
                )

            # chunk 0 must exist before step 0 reads it; the rest are paced
            # into the early steps' AllGather-wait windows
            for g in range(4):
                emit_gx_chunk(g, 0)
            gxq = [(g, c) for c in range(1, 8) for g in range(4)]

            # ---- d2 work queue ----------------------------------------
            d2q = []
            _hid_sb = {}

            def emit_d2_unit(j, m, mrows, vc):
                ps2 = d2_pp.tile([128, VC], F32, tag="d2")
                hidT, cols0 = _hid_sb[j]
                for k in range(KH):
                    nc.tensor.matmul(
                        ps2[:mrows, :], hidT[:, k, m * 128:m * 128 + mrows],
                        wd2_sb[:, k, vc * VC:(vc + 1) * VC],
                        start=(k == 0), stop=(k == KH - 1),
                    )
                osb = wpool.tile([128, VC], F16, tag="outsb")
                nc.vector.tensor_tensor(
                    out=osb[:mrows, :], in0=ps2[:mrows, :],
                    in1=bd2b_sb[:mrows, vc * VC:(vc + 1) * VC],
                    op=mybir.AluOpType.add,
                )
                r0 = cols0 + m * 128
                nc.sync.dma_start(
                    out=outp[r0:r0 + mrows, vc * VC:(vc + 1) * VC],
                    in_=osb[:mrows, :],
                )

            def emit_filler(nmax):
                done = 0
                while done < nmax and gxq:
                    emit_gx_chunk(*gxq.pop(0))
                    done += 1
                while done < nmax and d2q:
                    emit_d2_unit(*d2q.pop(0))
                    done += 1

            # ---- main recurrence loop ---------------------------------
            bi = 0  # next batch index
            for t in range(T):
                # full h_t from the AllGather, split into two half-loads so
                # the k=0..3 whh matmuls overlap the second half's stream-in
                hT = wpool.tile([128, KH, B], F16, tag="hT")
                nc.sync.dma_start(
                    out=hT[:, 0:4, :],
                    in_=hc_all[t].rearrange("(q s p) b -> p s q b", s=2, p=128)[:, 0, 0:4, :],
                )
                nc.sync.dma_start(
                    out=hT[:, 4:8, :],
                    in_=hc_all[t].rearrange("(q s p) b -> p s q b", s=2, p=128)[:, 0, 4:8, :],
                )

                ps = gates_pp.tile([128, 4 * B], F32, tag="gates")
                # inject Gx[t] (includes ctx part + biases), then accumulate whh
                nc.tensor.matmul(
                    ps[:], ident[:],
                    gxe_tiles[t // 8][:, t % 8, :, :].rearrange("p g b -> p (g b)"),
                    start=True, stop=False)
                # k-major so the first 16 matmuls consume only hT's first half
                for k in range(KH):
                    for g in range(4):
                        sl = ps[:, g * B:(g + 1) * B]
                        nc.tensor.matmul(sl, whh_sb[:, k, g, :], hT[:, k, :],
                                         start=False,
                                         stop=(g == 3 and k == KH - 1))

                # pointwise: gate order is [i|f|o|g]
                sg3 = epool.tile([128, 3 * B], F32, tag="sg3")
                nc.scalar.activation(sg3[:], ps[:, 0:3 * B], AF.Sigmoid)
                tang = epool.tile([128, B], F32, tag="tang")
                nc.scalar.activation(tang[:], ps[:, 3 * B:4 * B], AF.Tanh)
                t1 = epool.tile([128, B], F32, tag="t1")
                nc.vector.tensor_tensor(out=t1[:], in0=sg3[:, B:2 * B], in1=c_loc[:],
                                        op=mybir.AluOpType.mult)
                t2 = epool.tile([128, B], F32, tag="t2")
                nc.vector.tensor_tensor(out=t2[:], in0=sg3[:, 0:B], in1=tang[:],
                                        op=mybir.AluOpType.mult)
                c_loc = epool.tile([128, B], F32, tag="cloc")
                nc.vector.tensor_tensor(out=c_loc[:], in0=t1[:], in1=t2[:],
                                        op=mybir.AluOpType.add)
                tanc = epool.tile([128, B], F32, tag="tanc")
                nc.scalar.activation(tanc[:], c_loc[:], AF.Tanh)
                hc16 = epool.tile([128, 2, B], F16, tag="hc16")
                nc.vector.tensor_tensor(out=hc16[:, 0, :], in0=sg3[:, 2 * B:3 * B],
                                        in1=tanc[:], op=mybir.AluOpType.mult)
                nc.vector.tensor_copy(out=hc16[:, 1, :], in_=c_loc[:])

                nc.sync.dma_start(
                    out=hc_in[t + 1].rearrange("(s p) b -> p s b", p=128),
                    in_=hc16[:])
                nc.gpsimd.collective_compute(
                    "AllGather", mybir.AluOpType.bypass,
                    ins=[hc_in[t + 1]], outs=[hc_all[t + 1]],
                    replica_groups=rgroups,
                )

                # batch boundary: d1 + hid AllGather, then queue d2 units
                if bi < NB and t == BATCHES[bi][0] + BATCHES[bi][1] - 1:
                    s0, ns = BATCHES[bi]
                    ncols = ns * B
                    psd1 = d1_pp.tile([128, 512], F32, tag="d1")
                    for k in range(KD1):
                        rhs = wpool.tile([128, 8 * B], F16, tag="d1rhs")
                        nc.sync.dma_start(
                            out=rhs[:, :ncols],
                            in_=hc_all[s0 + 1:s0 + ns + 1,
                                       k * 128:(k + 1) * 128, :].rearrange(
                                       "s p b -> p s b"),
                        )
                        nc.tensor.matmul(psd1[:, :ncols], wd1_sb[:, k, :],
                                         rhs[:, :ncols],
                                         start=(k == 0), stop=(k == KD1 - 1))
                    hloc = wpool.tile([128, 512], F16, tag="hloc")
                    nc.scalar.activation(hloc[:, :ncols], psd1[:, :ncols],
                                         AF.Tanh, bias=bd1_sb[:])
                    nc.sync.dma_start(out=hid_in[bi, :, :ncols],
                                      in_=hloc[:, :ncols])
                    nc.gpsimd.collective_compute(
                        "AllGather", mybir.AluOpType.bypass,
                        ins=[hid_in[bi]], outs=[hid_all[bi]],
                        replica_groups=rgroups,
                    )
                    hsb = wpool.tile([128, KH, 512], F16, tag="hidT", bufs=2)
                    nc.sync.dma_start(
                        out=hsb[:, :, :ncols],
                        in_=hid_all[bi, :, :ncols].rearrange(
                            "(k p) n -> p k n", p=128),
                    )
                    _hid_sb[bi] = (hsb, s0 * B)
                    for m in range((ncols + 127) // 128):
                        mrows = min(128, ncols - m * 128)
                        for vc in range(VS // VC):
                            d2q.append((bi, m, mrows, vc))
                    bi += 1

                # paced filler: ~4.5 units/step so the queue neither
                # starves the PE nor drains before the last batches land
                emit_filler(5 if t % 2 else 4)

            while gxq:
                emit_gx_chunk(*gxq.pop(0))
            while d2q:
                emit_d2_unit(*d2q.pop(0))

    nc.finalize()
    return nc


GATE_ORDER = [0, 1, 3, 2]  # reference i,f,g,o -> kernel [i|f|o|g]


def _prep_inputs(seq, context, emb, W_ih, b_ih, W_hh, b_hh, W_initS, b_initS,
                 W_initC, b_initC, W_d1, b_d1, W_d2, b_d2):
    """Host-side layout prep: transposes, fp16 casts, per-core sharding."""
    f16, f32 = np.float16, np.float32
    seq = np.asarray(seq)
    context = np.asarray(context, f32)
    emb = np.asarray(emb, f32)

    # emb[seq].T all steps: [KE, 128, T*B] (column index = t*B + b)
    g = emb[seq.reshape(-1)].reshape(T * B, E).T        # [E, T*B]
    embt = np.ascontiguousarray(g.reshape(KE, 128, T * B)).astype(f16)

    ctxT = np.ascontiguousarray(context.T)              # [CTX, B]
    ctx16 = ctxT.reshape(KH, 128, B).astype(f16)

    bsum = (np.asarray(b_ih, f32) + np.asarray(b_hh, f32))  # [4H]

    W_ihc = np.asarray(W_ih, f32)[:, :CTX]              # [4H, CTX]
    W_ihe = np.asarray(W_ih, f32)[:, CTX:]              # [4H, E]
    W_hh = np.asarray(W_hh, f32)
    W_d1 = np.asarray(W_d1, f32)
    W_d2 = np.asarray(W_d2, f32)

    # d1 row permutation to match AllGather layout [h_q; c_q interleaved]
    perm = np.empty(2 * H, np.int64)
    for q in range(R):
        perm[256 * q:256 * q + 128] = np.arange(128 * q, 128 * (q + 1))
        perm[256 * q + 128:256 * (q + 1)] = H + np.arange(128 * q, 128 * (q + 1))
    W_d1T_perm = W_d1.T[perm, :]                        # [2H, H]

    maps = []
    for r in range(R):
        rows = lambda g_: slice(1024 * g_ + 128 * r, 1024 * g_ + 128 * (r + 1))

        def gate_tiles(W, KT):
            # [KT, 4, 128(k), 128(m)]: W rows = gate-chunk rows of core r,
            # gates reordered to [i|f|o|g]
            a = np.empty((KT, 4, 128, 128), f32)
            for gi in range(4):
                Wg = W[rows(GATE_ORDER[gi])]            # [128, KT*128]
                a[:, gi] = Wg.reshape(128, KT, 128).transpose(1, 2, 0)
            return a.astype(f16)

        whh_r = gate_tiles(W_hh, KH)
        wihc_r = gate_tiles(W_ihc, KH)
        wihe_r = gate_tiles(W_ihe, KE)
        bg_r = np.stack([bsum[rows(GATE_ORDER[gi])]
                         for gi in range(4)]).astype(f32)  # [4,128]

        hcrows = slice(128 * r, 128 * (r + 1))
        winits_r = np.ascontiguousarray(
            np.asarray(W_initS, f32)[hcrows].T.reshape(KH, 128, 128)).astype(f16)
        winitc_r = np.ascontiguousarray(
            np.asarray(W_initC, f32)[hcrows].T.reshape(KH, 128, 128)).astype(f16)
        binits_r = np.asarray(b_initS, f32)[hcrows].copy()
        binitc_r = np.asarray(b_initC, f32)[hcrows].copy()

        wd1_r = np.ascontiguousarray(
            W_d1T_perm[:, hcrows].reshape(KD1, 128, 128)).astype(f16)
        bd1_r = np.asarray(b_d1, f32)[hcrows].copy()

        vsl = slice(VS * r, VS * (r + 1))
        wd2_r = np.ascontiguousarray(
            W_d2[vsl].T.reshape(KH, 128, VS)).astype(f16)
        bd2b_r = np.broadcast_to(
            np.asarray(b_d2, f32)[vsl], (128, VS)).astype(f16).copy()

        maps.append({
            "ctx16": ctx16, "embt": embt,
            "whh": whh_r, "wihc": wihc_r, "wihe": wihe_r, "bg": bg_r,
            "binits": binits_r, "binitc": binitc_r,
            "winits": winits_r, "winitc": winitc_r,
            "wd1": wd1_r, "bd1": bd1_r,
            "wd2": wd2_r, "bd2b": bd2b_r,
        })
    return maps


def kernel(**inputs):
    inputs.pop("mode", None)
    in_maps = _prep_inputs(**{k: np.asarray(v) for k, v in inputs.items()})
    if "nc" not in _CACHE:
        _CACHE["nc"] = _build_program()
    res = run_bass_kernel_spmd(_CACHE["nc"], in_maps, list(range(R)))
    _CACHE["last_res"] = res
    if getattr(res, "exec_time_ns", None):
        print(f"[profile] exec_time_ns: {res.exec_time_ns}")
    shards = [res.results[r]["outp"] for r in range(R)]       # each [T*B, VS] f16
    out = np.concatenate(shards, axis=1).astype(np.float32)   # [T*B, V]
    return out.reshape(T, B, V)


def timed_runs(inputs, n=6):
    """Test-only helper: execute the compiled program n times on device-
    resident inputs and return per-iteration wall times (seconds)."""
    import jax
    import jax.numpy as jnp
    from jax.sharding import Mesh, PartitionSpec, NamedSharding
    from jax.experimental.shard_map import shard_map
    from concourse import bass2jax
    import concourse.mybir as mybir_

    inputs = {k: np.asarray(v) for k, v in inputs.items()}
    inputs.pop("mode", None)
    in_maps = _prep_inputs(**inputs)
    if "nc" not in _CACHE:
        _CACHE["nc"] = _build_program()
    nc = _CACHE["nc"]
    bass2jax.install_neuronx_cc_hook()

    partition_name = nc.partition_id_tensor.name if nc.partition_id_tensor else None
    in_names, out_names, out_avals = [], [], []
    for alloc in nc.m.functions[0].allocations:
        if not isinstance(alloc, mybir_.MemoryLocationSet):
            continue
        name = alloc.memorylocations[0].name
        if alloc.kind == "ExternalInput":
            if name != partition_name:
                in_names.append(name)
        elif alloc.kind == "ExternalOutput":
            out_names.append(name)
            out_avals.append(
                jax.core.ShapedArray(tuple(alloc.tensor_shape),
                                     mybir_.dt.np(alloc.dtype)))

    all_in_names = in_names + out_names
    if partition_name is not None:
        all_in_names = all_in_names + [partition_name]

    def _body(*args):
        operands = list(args)
        if partition_name is not None:
            operands.append(bass2jax.partition_id_tensor())
        outs = bass2jax._bass_exec_p.bind(
            *operands, out_avals=tuple(out_avals),
            in_names=tuple(all_in_names),
            out_names=tuple(out_names),
            lowering_input_output_aliases=(),
            sim_require_finite=True, sim_require_nnan=True, nc=nc,
        )
        return tuple(outs)

    devices = jax.devices()[:R]
    mesh = Mesh(np.asarray(devices), ("core",))
    nspec = (PartitionSpec("core"),) * (len(in_names) + len(out_names))
    sharded = jax.jit(shard_map(_body, mesh=mesh, in_specs=nspec,
                                out_specs=(PartitionSpec("core"),) * len(out_names),
                                check_rep=False), keep_unused=True)

    concat_in = [
        jax.device_put(
            np.concatenate([np.asarray(in_maps[c][nm]) for c in range(R)], axis=0),
            NamedSharding(mesh, PartitionSpec("core")))
        for nm in in_names
    ]
    zero_fn = jax.jit(
        lambda: tuple(
            jnp.zeros((R * av.shape[0], *av.shape[1:]), av.dtype)
            for av in out_avals),
        out_shardings=tuple(NamedSharding(mesh, PartitionSpec("core"))
                            for _ in out_avals))
    zeros = [jax.block_until_ready(z) for z in zero_fn()]

    times = []
    for _ in range(n):
        t0 = time.time()
        outs = sharded(*concat_in, *zeros)
        jax.block_until_ready(outs)
        times.append(time.time() - t0)
    return times


if __name__ == "__main__":
    rng = np.random.default_rng(0)
    ins = {
        "seq": rng.integers(0, V, (T, B)).astype(np.int32),
        "context": rng.standard_normal((B, CTX)).astype(np.float32),
        "emb": (rng.standard_normal((V, E)) * 0.02).astype(np.float32),
        "W_ih": (rng.standard_normal((4 * H, E + CTX)) / np.sqrt(E + CTX)).astype(np.float32),
        "b_ih": np.zeros(4 * H, np.float32),
        "W_hh": (rng.standard_normal((4 * H, H)) / np.sqrt(H)).astype(np.float32),
        "b_hh": np.zeros(4 * H, np.float32),
        "W_initS": (rng.standard_normal((H, CTX)) / np.sqrt(CTX)).astype(np.float32),
        "b_initS": np.zeros(H, np.float32),
        "W_initC": (rng.standard_normal((H, CTX)) / np.sqrt(CTX)).astype(np.float32),
        "b_initC": np.zeros(H, np.float32),
        "W_d1": (rng.standard_normal((H, 2 * H)) / np.sqrt(2 * H)).astype(np.float32),
        "b_d1": np.zeros(H, np.float32),
        "W_d2": (rng.standard_normal((V, H)) / np.sqrt(H)).astype(np.float32),
        "b_d2": np.zeros(V, np.float32),
        "mode": 1,
    }
    out = kernel(**ins)
    print("kernel output", out.shape, out.dtype, float(np.abs(out).max()))
